# revision 2
# baseline (speedup 1.0000x reference)
"""Trainium2 Bass kernel for nn_AttentionContextEncoder (gnn_message_passing).

reference:
  ents = ctx.T.reshape(B, 7, 4)
  prop_emb = relu(ents @ w_prop + b_prop)                      # [B,7,128]
  diffs[b,i,j,:] = ents[b,i,:] - ents[b,j,:]
  dist = sqrt(diffs[...,0]^2 + diffs[...,1]^2)
  rel = relu(concat([diffs, dist]) @ w_rel + b_rel)            # [B,7,7,128]
  rel_emb = sum_{j != i} rel[:, i, j, :]                       # [B,7,128]
  out = concat([prop_emb, rel_emb], -1)                        # [B,7,256]

v3 design (data-parallel over 8 cores, B=2048/core):
- Same math as v2: host-built gather matrix G (diff combos), squared-dist
  reduction R, and wp/wm/wq weight images with bias folded in via a ones
  contraction row. Per directed pair one K=6 matmul per [128,1024] PSUM
  half-unit; relu+sum drains split across ACT (relu->r tiles), DVE
  (fused relu-accumulate stt chains) and GpSimd (bf16 SBUF combines).
- v3 scheduling fixes (the v2 span was PE-bound at cold-isolated MMs):
  * pairs issue in 2-pair waves on different PE row strips, MMs
    interleaved at chunk level -> tile_position concurrency instead of
    serial 427ns/MM cold streams.
  * each wave feeds one ACT pair + one DVE pair so both drain engines
    stream continuously instead of alternating bursts.
  * all staging/input/output DMAs issue from the Sync queue (v2 burned
    ~15us of GpSimd+Scalar time on dma_start issue).
  * prop6 pages staged straight from DRAM ctx; ones rows from a host
    constant: G loses its prop columns (112->84 rows of cmp).
  * combines/folds are full [128,2048] GpSimd ops; fold/prop engine
    splits tuned so ACT/DVE/GpS land ~balanced.
- Output bf16 [2,7,128,B] per core; host concatenates, converts to f32,
  and transposes.
"""
import numpy as np
import ml_dtypes
from contextlib import ExitStack

import concourse.bass as bass
import concourse.bacc as bacc
import concourse.mybir as mybir
import concourse.tile as tile
from concourse.bass_utils import run_bass_kernel_spmd

F32 = mybir.dt.float32
BF16 = mybir.dt.bfloat16
AF = mybir.ActivationFunctionType
ALU = mybir.AluOpType

NUM_ENT = 7
DIM_ENT = 4
H = 128
B_TOTAL = 16384
N_CORES = 8
B = B_TOTAL // N_CORES          # 2048 per core
HB = B // 2                     # 1024 half-batch (one [128,1024] PSUM unit)

# ---- pair table: K7 edge-coloring so each target's 6 pairs spread over
# 4 strips (<=2 per strip). class c = {(i,j): i+j = c mod 7}; strips
# take classes {0,1},{2,3},{4,5},{6} -> sizes {6,6,6,3}.
_CLS = [[] for _ in range(7)]
for i in range(NUM_ENT):
    for j in range(i + 1, NUM_ENT):
        _CLS[(i + j) % 7].append((i, j))
_STRIP_PAIRS = [_CLS[0] + _CLS[1], _CLS[2] + _CLS[3], _CLS[4] + _CLS[5], _CLS[6]]
PAIRS = [p for sp in _STRIP_PAIRS for p in sp]       # ordinal = strip-major
STRIP_NP = [len(sp) for sp in _STRIP_PAIRS]          # [6, 6, 6, 3]
STRIP_START = [0, 6, 12, 18]
PAIR_SG = {}
for s in range(4):
    for g in range(STRIP_NP[s]):
        PAIR_SG[STRIP_START[s] + g] = (s, g)
PAIR_IDX = {PAIRS[k]: k for k in range(21)}

# prop t -> (strip, page) in prop6
PROP_SG = {t: (t % 4, t // 4) for t in range(NUM_ENT)}

# targets whose prop relu runs on DVE (tensor_single_scalar) vs ACT
PROP_DVE = {4, 5, 6}
# targets whose final acc += c fold runs on DVE (others on GpSimd)
FOLD_DVE = {5, 6}


def _ordered_pairs(t):
    """t's partners ordered round-robin over strips for PE concurrency."""
    by_strip = [[] for _ in range(4)]
    for j in range(NUM_ENT):
        if j == t:
            continue
        a, b_ = (t, j) if t < j else (j, t)
        s, _ = PAIR_SG[PAIR_IDX[(a, b_)]]
        by_strip[s].append(j)
    order = []
    r = 0
    while len(order) < 6:
        for s in range(4):
            if len(by_strip[s]) > r:
                order.append(by_strip[s][r])
        r += 1
    return order


def build_constants(w_prop, b_prop, w_rel, b_rel):
    """Host-side constant arrays (bf16)."""
    bf = ml_dtypes.bfloat16
    # G: [28, 84]: col 21c+k = diff dim c of pair k
    G = np.zeros((NUM_ENT * DIM_ENT, 84), np.float32)
    for k, (i, j) in enumerate(PAIRS):
        for c in range(DIM_ENT):
            G[4 * i + c, 21 * c + k] = 1.0
            G[4 * j + c, 21 * c + k] = -1.0
    # R: [42, 21]: col k sums sq rows k (dd0^2) and 21+k (dd1^2)
    R = np.zeros((42, 21), np.float32)
    for k in range(21):
        R[k, k] = 1.0
        R[21 + k, k] = 1.0
    # weight images [128, 128]: per strip s rows 32s+r:
    #   r=0..3: +/-w_rel[0:4]; r=4: w_rel[4]; r=5: b_rel   (wp / wm)
    #   wq: r=0..3: w_prop; r=4: b_prop
    wp = np.zeros((H, H), np.float32)
    wm = np.zeros((H, H), np.float32)
    wq = np.zeros((H, H), np.float32)
    for s in range(4):
        r0 = 32 * s
        wp[r0:r0 + 4, :] = w_rel[0:4]
        wp[r0 + 4, :] = w_rel[4]
        wp[r0 + 5, :] = b_rel
        wm[r0:r0 + 4, :] = -w_rel[0:4]
        wm[r0 + 4, :] = w_rel[4]
        wm[r0 + 5, :] = b_rel
        wq[r0:r0 + 4, :] = w_prop
        wq[r0 + 4, :] = b_prop
    ones6 = np.ones((6, B), np.float32)
    return {
        "gmat": G.astype(bf), "rmat": R.astype(bf),
        "wpimg": wp.astype(bf), "wmimg": wm.astype(bf), "wqimg": wq.astype(bf),
        "onesb": ones6.astype(bf),
    }


def build():
    nc = bacc.Bacc("TRN2", target_bir_lowering=False, debug=False,
                   num_devices=N_CORES)
    ctxb_d = nc.dram_tensor("ctxb", [NUM_ENT * DIM_ENT, B], BF16,
                            kind="ExternalInput").ap()
    gmat_d = nc.dram_tensor("gmat", [NUM_ENT * DIM_ENT, 84], BF16,
                            kind="ExternalInput").ap()
    rmat_d = nc.dram_tensor("rmat", [42, 21], BF16, kind="ExternalInput").ap()
    wp_d = nc.dram_tensor("wpimg", [H, H], BF16, kind="ExternalInput").ap()
    wm_d = nc.dram_tensor("wmimg", [H, H], BF16, kind="ExternalInput").ap()
    wq_d = nc.dram_tensor("wqimg", [H, H], BF16, kind="ExternalInput").ap()
    ones_d = nc.dram_tensor("onesb", [6, B], BF16, kind="ExternalInput").ap()
    out_d = nc.dram_tensor("out", [2, NUM_ENT, H, B], BF16,
                           kind="ExternalOutput").ap()

    with tile.TileContext(nc) as tc, ExitStack() as ctx:
        stat = ctx.enter_context(tc.tile_pool(name="stat", bufs=1))
        rp = ctx.enter_context(tc.tile_pool(name="rp", bufs=6))
        accp = ctx.enter_context(tc.tile_pool(name="accp", bufs=3))
        poutp = ctx.enter_context(tc.tile_pool(name="poutp", bufs=3))
        cbp = ctx.enter_context(tc.tile_pool(name="cbp", bufs=2))

        psl = ctx.enter_context(tc.tile_pool(name="psl", bufs=4, space="PSUM"))

        # ---------- inputs (all issued from sync queue) ----------
        ctxb = stat.tile([NUM_ENT * DIM_ENT, B], BF16)
        nc.sync.dma_start(ctxb[:], ctxb_d[:])
        gm = stat.tile([NUM_ENT * DIM_ENT, 84], BF16)
        nc.sync.dma_start(gm[:], gmat_d[:])
        rm = stat.tile([42, 21], BF16)
        nc.sync.dma_start(rm[:], rmat_d[:])
        wpimg = stat.tile([H, H], BF16)
        nc.sync.dma_start(wpimg[:], wp_d[:])
        wmimg = stat.tile([H, H], BF16)
        nc.sync.dma_start(wmimg[:], wm_d[:])
        wqimg = stat.tile([H, H], BF16)
        nc.sync.dma_start(wqimg[:], wq_d[:])
        onesb = stat.tile([6, B], BF16)
        nc.sync.dma_start(onesb[:], ones_d[:])

        # prop6 pages straight from DRAM ctx + ones rows
        prop6 = stat.tile([H, 2, B], BF16)
        for t in range(NUM_ENT):
            ps_, pg_ = PROP_SG[t]
            nc.sync.dma_start(prop6[32 * ps_:32 * ps_ + 4, pg_, :],
                              ctxb_d[4 * t:4 * t + 4, :])
        for s in range(4):
            nc.sync.dma_start(prop6[32 * s + 4:32 * s + 5, 0:2, :],
                              onesb[0:2, :])

        # ---------- prep: G-matmul -> cmpb, sq, R-matmul -> dist ----------
        cmpb = stat.tile([84, B], BF16)
        for h in range(2):
            cslot = psl.tile([84, HB], F32, tag="slot", name="cslot")
            for c in range(2):
                nc.tensor.matmul(cslot[:, 512 * c:512 * c + 512],
                                 gm[:, :],
                                 ctxb[:, HB * h + 512 * c:HB * h + 512 * c + 512],
                                 start=True, stop=True, tile_position=(0, 0))
            nc.scalar.copy(cmpb[:, HB * h:HB * h + HB], cslot[:])
        sq = stat.tile([42, B], BF16)
        nc.vector.tensor_mul(sq[0:42, :], cmpb[0:42, :], cmpb[0:42, :])
        distb = stat.tile([21, B], BF16)
        for h in range(2):
            dslot = psl.tile([21, HB], F32, tag="slot", name="dslot")
            for c in range(2):
                nc.tensor.matmul(dslot[:, 512 * c:512 * c + 512],
                                 rm[0:42, :],
                                 sq[0:42, HB * h + 512 * c:HB * h + 512 * c + 512],
                                 start=True, stop=True, tile_position=(0, 0))
            nc.scalar.activation(distb[:, HB * h:HB * h + HB], dslot[:], AF.Sqrt)

        # ---------- staging (sync queue only) ----------
        # rhs6 [128, 6, B]: strip s rows 32s+0..3 diffs, +4 dist, +5 ones
        rhs6 = stat.tile([H, 6, B], BF16)
        for s in range(4):
            k0, np_ = STRIP_START[s], STRIP_NP[s]
            for c in range(DIM_ENT):
                nc.sync.dma_start(rhs6[32 * s + c:32 * s + c + 1, 0:np_, :],
                                  cmpb[21 * c + k0:21 * c + k0 + np_, :])
            nc.sync.dma_start(rhs6[32 * s + 4:32 * s + 5, 0:np_, :],
                              distb[k0:k0 + np_, :])
            nc.sync.dma_start(rhs6[32 * s + 5:32 * s + 6, 0:6, :],
                              onesb[0:6, :])

        # ---------- main ----------
        def pair_mm_chunk(t, j, h, c, slot):
            """one 512-col matmul for directed pair (t -> j) into slot."""
            a, b_ = (t, j) if t < j else (j, t)
            s, g = PAIR_SG[PAIR_IDX[(a, b_)]]
            img = wpimg if t < j else wmimg
            nc.tensor.matmul(
                slot[:, 512 * c:512 * c + 512],
                img[32 * s:32 * s + 6, :],
                rhs6[32 * s:32 * s + 6, g,
                     HB * h + 512 * c:HB * h + 512 * c + 512],
                start=True, stop=True, tile_position=(32 * s, 0))

        def prop_mm(t, h):
            ps_, pg_ = PROP_SG[t]
            slot = psl.tile([H, HB], F32, tag="slot", name="pslot")
            for c in range(2):
                nc.tensor.matmul(
                    slot[:, 512 * c:512 * c + 512],
                    wqimg[32 * ps_:32 * ps_ + 5, :],
                    prop6[32 * ps_:32 * ps_ + 5, pg_,
                          HB * h + 512 * c:HB * h + 512 * c + 512],
                    start=True, stop=True, tile_position=(32 * ps_, 0))
            return slot

        for t in range(NUM_ENT):
            order = _ordered_pairs(t)
            # wave w = (order[2w] -> ACT relu into r-tile, order[2w+1] -> DVE
            # fused stt). chain in1: wave0 uses r0; combine c = r1 + r2 on
            # GpSimd; final fold acc += c.
            acc = accp.tile([H, B], BF16, tag="acc", name="acc")
            rbufs = []
            for w in range(3):
                pa, pd = order[2 * w], order[2 * w + 1]
                r = rp.tile([H, B], BF16, tag="r", name="r")
                rbufs.append(r)
                for h in range(2):
                    slotA = psl.tile([H, HB], F32, tag="slot", name="aslot")
                    slotB = psl.tile([H, HB], F32, tag="slot", name="dslot2")
                    for c in range(2):
                        pair_mm_chunk(t, pa, h, c, slotA)
                        pair_mm_chunk(t, pd, h, c, slotB)
                    nc.scalar.activation(r[:, HB * h:HB * h + HB],
                                         slotA[:], AF.Relu)
                    in1 = (rbufs[0][:, HB * h:HB * h + HB] if w == 0
                           else acc[:, HB * h:HB * h + HB])
                    nc.vector.scalar_tensor_tensor(
                        acc[:, HB * h:HB * h + HB], slotB[:], 0.0, in1,
                        op0=ALU.max, op1=ALU.add)
                if w == 1:
                    # prop mini-wave between waves 1 and 2
                    pout = poutp.tile([H, B], BF16, tag="pout", name="pout")
                    for h in range(2):
                        slot = prop_mm(t, h)
                        if t in PROP_DVE:
                            nc.vector.tensor_single_scalar(
                                pout[:, HB * h:HB * h + HB], slot[:], 0.0,
                                op=ALU.max)
                        else:
                            nc.scalar.activation(pout[:, HB * h:HB * h + HB],
                                                 slot[:], AF.Relu)
                    nc.sync.dma_start(out_d[0, t, :, :], pout[:])
            # combine the two non-chained ACT tiles, then fold into acc
            cb = cbp.tile([H, B], BF16, tag="c1", name="c1")
            nc.gpsimd.tensor_add(cb[:], rbufs[1][:], rbufs[2][:])
            if t in FOLD_DVE:
                nc.vector.tensor_add(acc[:], acc[:], cb[:])
            else:
                nc.gpsimd.tensor_add(acc[:], acc[:], cb[:])
            nc.sync.dma_start(out_d[1, t, :, :], acc[:])

    nc.compile()
    return nc


_NC_CACHE = None


def _get_nc():
    global _NC_CACHE
    if _NC_CACHE is None:
        _NC_CACHE = build()
    return _NC_CACHE


def run(ctx, w_prop, b_prop, w_rel, b_rel, trace=False):
    bf = ml_dtypes.bfloat16
    ctx = np.asarray(ctx, dtype=np.float32)
    nc = _get_nc()
    shared = build_constants(np.asarray(w_prop, np.float32),
                             np.asarray(b_prop, np.float32),
                             np.asarray(w_rel, np.float32),
                             np.asarray(b_rel, np.float32))
    in_maps = []
    for c in range(N_CORES):
        m = dict(shared)
        m["ctxb"] = np.ascontiguousarray(ctx[:, c * B:(c + 1) * B]).astype(bf)
        in_maps.append(m)
    res = run_bass_kernel_spmd(nc, in_maps, core_ids=list(range(N_CORES)),
                               trace=trace)
    shards = [np.asarray(res.results[c]["out"]).astype(np.float32)
              for c in range(N_CORES)]
    full = np.concatenate(shards, axis=3)                     # [2,7,128,16384]
    out = np.transpose(full, (3, 1, 0, 2)).reshape(B_TOTAL, NUM_ENT, 2 * H)
    return np.ascontiguousarray(out), res


def kernel(ctx, w_prop, b_prop, w_rel, b_rel):
    return run(ctx, w_prop, b_prop, w_rel, b_rel)[0]


# revision 5
# speedup vs baseline: 1.0751x; 1.0751x over previous
"""Trainium2 Bass kernel for nn_AttentionContextEncoder (gnn_message_passing).

reference:
  ents = ctx.T.reshape(B, 7, 4)
  prop_emb = relu(ents @ w_prop + b_prop)                      # [B,7,128]
  diffs[b,i,j,:] = ents[b,i,:] - ents[b,j,:]
  dist = sqrt(diffs[...,0]^2 + diffs[...,1]^2)
  rel = relu(concat([diffs, dist]) @ w_rel + b_rel)            # [B,7,7,128]
  rel_emb = sum_{j != i} rel[:, i, j, :]                       # [B,7,128]
  out = concat([prop_emb, rel_emb], -1)                        # [B,7,256]

v4 design (data-parallel over 8 cores, B=2048/core):
- Math as v2/v3: host G (diff combos), R (sq-dist reduce), wp/wm/wq
  images with bias folded via a ones contraction row; one K=6 matmul
  per directed pair per [128,1024] PSUM half; drains split across
  ACT (relu->r), DVE (fused relu-accumulate stt chains), GpSimd
  (bf16 combines/folds).
- v4 scheduling:
  * prop phase runs FIRST (needs only DRAM-staged prop6, not rhs6),
    so prop matmuls+drains overlap the rhs6 staging latency.
  * pair matmuls issue as a global stream in 4-pair waves across the
    4 PE row strips, chunk-interleaved -> tile_position concurrency.
  * every wave carries 2 ACT pairs + 2 DVE pairs so both drain
    engines stream continuously.
  * staging DMAs spread over sync+gpsimd+tensor queues; outputs on
    sync.
  * GpSimd combines per-half (fed by the two earliest relus of each
    target), folds on GpSimd except the last target (DVE) to cut the
    tail.
- Output bf16 [2,7,128,B] per core; host concatenates, converts to
  f32, and transposes.
"""
import numpy as np
import ml_dtypes
from contextlib import ExitStack

import concourse.bass as bass
import concourse.bacc as bacc
import concourse.mybir as mybir
import concourse.tile as tile
from concourse.bass_utils import run_bass_kernel_spmd

F32 = mybir.dt.float32
BF16 = mybir.dt.bfloat16
AF = mybir.ActivationFunctionType
ALU = mybir.AluOpType

NUM_ENT = 7
DIM_ENT = 4
H = 128
B_TOTAL = 16384
N_CORES = 8
B = B_TOTAL // N_CORES          # 2048 per core
HB = B // 2                     # 1024 half-batch (one [128,1024] PSUM unit)

# ---- pair table: K7 edge-coloring so each target's 6 pairs spread over
# 4 strips (<=2 per strip). class c = {(i,j): i+j = c mod 7}; strips
# take classes {0,1},{2,3},{4,5},{6} -> sizes {6,6,6,3}.
_CLS = [[] for _ in range(7)]
for i in range(NUM_ENT):
    for j in range(i + 1, NUM_ENT):
        _CLS[(i + j) % 7].append((i, j))
_STRIP_PAIRS = [_CLS[0] + _CLS[1], _CLS[2] + _CLS[3], _CLS[4] + _CLS[5], _CLS[6]]
PAIRS = [p for sp in _STRIP_PAIRS for p in sp]       # ordinal = strip-major
STRIP_NP = [len(sp) for sp in _STRIP_PAIRS]          # [6, 6, 6, 3]
STRIP_START = [0, 6, 12, 18]
PAIR_SG = {}
for s in range(4):
    for g in range(STRIP_NP[s]):
        PAIR_SG[STRIP_START[s] + g] = (s, g)
PAIR_IDX = {PAIRS[k]: k for k in range(21)}

# prop t -> (strip, page) in prop6
PROP_SG = {t: (t % 4, t // 4) for t in range(NUM_ENT)}


def _ordered_pairs(t):
    """t's partners ordered round-robin over strips for PE concurrency."""
    by_strip = [[] for _ in range(4)]
    for j in range(NUM_ENT):
        if j == t:
            continue
        a, b_ = (t, j) if t < j else (j, t)
        s, _ = PAIR_SG[PAIR_IDX[(a, b_)]]
        by_strip[s].append(j)
    order = []
    r = 0
    while len(order) < 6:
        for s in range(4):
            if len(by_strip[s]) > r:
                order.append(by_strip[s][r])
        r += 1
    return order


def build_constants(w_prop, b_prop, w_rel, b_rel):
    """Host-side constant arrays (bf16)."""
    bf = ml_dtypes.bfloat16
    # G: [28, 84]: col 21c+k = diff dim c of pair k
    G = np.zeros((NUM_ENT * DIM_ENT, 84), np.float32)
    for k, (i, j) in enumerate(PAIRS):
        for c in range(DIM_ENT):
            G[4 * i + c, 21 * c + k] = 1.0
            G[4 * j + c, 21 * c + k] = -1.0
    # R: [42, 21]: col k sums sq rows k (dd0^2) and 21+k (dd1^2)
    R = np.zeros((42, 21), np.float32)
    for k in range(21):
        R[k, k] = 1.0
        R[21 + k, k] = 1.0
    # weight images [128, 128]: per strip s rows 32s+r:
    #   r=0..3: +/-w_rel[0:4]; r=4: w_rel[4]; r=5: b_rel   (wp / wm)
    #   wq: r=0..3: w_prop; r=4: b_prop
    wp = np.zeros((H, H), np.float32)
    wm = np.zeros((H, H), np.float32)
    wq = np.zeros((H, H), np.float32)
    for s in range(4):
        r0 = 32 * s
        wp[r0:r0 + 4, :] = w_rel[0:4]
        wp[r0 + 4, :] = w_rel[4]
        wp[r0 + 5, :] = b_rel
        wm[r0:r0 + 4, :] = -w_rel[0:4]
        wm[r0 + 4, :] = w_rel[4]
        wm[r0 + 5, :] = b_rel
        wq[r0:r0 + 4, :] = w_prop
        wq[r0 + 4, :] = b_prop
    ones6 = np.ones((6, B), np.float32)
    return {
        "gmat": G.astype(bf), "rmat": R.astype(bf),
        "wpimg": wp.astype(bf), "wmimg": wm.astype(bf), "wqimg": wq.astype(bf),
        "onesb": ones6.astype(bf),
    }


def build():
    nc = bacc.Bacc("TRN2", target_bir_lowering=False, debug=False,
                   num_devices=N_CORES)
    ctxb_d = nc.dram_tensor("ctxb", [NUM_ENT * DIM_ENT, B], BF16,
                            kind="ExternalInput").ap()
    gmat_d = nc.dram_tensor("gmat", [NUM_ENT * DIM_ENT, 84], BF16,
                            kind="ExternalInput").ap()
    rmat_d = nc.dram_tensor("rmat", [42, 21], BF16, kind="ExternalInput").ap()
    wp_d = nc.dram_tensor("wpimg", [H, H], BF16, kind="ExternalInput").ap()
    wm_d = nc.dram_tensor("wmimg", [H, H], BF16, kind="ExternalInput").ap()
    wq_d = nc.dram_tensor("wqimg", [H, H], BF16, kind="ExternalInput").ap()
    ones_d = nc.dram_tensor("onesb", [6, B], BF16, kind="ExternalInput").ap()
    out_d = nc.dram_tensor("out", [2, NUM_ENT, H, B], BF16,
                           kind="ExternalOutput").ap()

    with tile.TileContext(nc) as tc, ExitStack() as ctx:
        stat = ctx.enter_context(tc.tile_pool(name="stat", bufs=1))
        rp = ctx.enter_context(tc.tile_pool(name="rp", bufs=9))
        accp = ctx.enter_context(tc.tile_pool(name="accp", bufs=3))
        poutp = ctx.enter_context(tc.tile_pool(name="poutp", bufs=4))
        cbp = ctx.enter_context(tc.tile_pool(name="cbp", bufs=3))

        psl = ctx.enter_context(tc.tile_pool(name="psl", bufs=4, space="PSUM"))

        # ---------- inputs ----------
        ctxb = stat.tile([NUM_ENT * DIM_ENT, B], BF16)
        nc.sync.dma_start(ctxb[:], ctxb_d[:])
        gm = stat.tile([NUM_ENT * DIM_ENT, 84], BF16)
        nc.sync.dma_start(gm[:], gmat_d[:])
        wqimg = stat.tile([H, H], BF16)
        nc.sync.dma_start(wqimg[:], wq_d[:])
        onesb = stat.tile([6, B], BF16)
        nc.sync.dma_start(onesb[:], ones_d[:])
        rm = stat.tile([42, 21], BF16)
        nc.gpsimd.dma_start(rm[:], rmat_d[:])
        wpimg = stat.tile([H, H], BF16)
        nc.gpsimd.dma_start(wpimg[:], wp_d[:])
        wmimg = stat.tile([H, H], BF16)
        nc.gpsimd.dma_start(wmimg[:], wm_d[:])

        # prop6 pages straight from DRAM ctx + ones rows
        prop6 = stat.tile([H, 2, B], BF16)
        for t in range(NUM_ENT):
            ps_, pg_ = PROP_SG[t]
            eng = nc.sync if t % 2 == 0 else nc.gpsimd
            eng.dma_start(prop6[32 * ps_:32 * ps_ + 4, pg_, :],
                          ctxb_d[4 * t:4 * t + 4, :])
        for s in range(4):
            nc.gpsimd.dma_start(prop6[32 * s + 4:32 * s + 5, 0:2, :],
                                onesb[0:2, :])

        # ---------- prep matmuls: G -> cmpb ----------
        cmpb = stat.tile([84, B], BF16)
        for h in range(2):
            cslot = psl.tile([84, HB], F32, tag="slot", name="cslot")
            for c in range(2):
                nc.tensor.matmul(cslot[:, 512 * c:512 * c + 512],
                                 gm[:, :],
                                 ctxb[:, HB * h + 512 * c:HB * h + 512 * c + 512],
                                 start=True, stop=True, tile_position=(0, 0))
            nc.scalar.copy(cmpb[:, HB * h:HB * h + HB], cslot[:])

        # ---------- prop phase (overlaps rhs6 staging below) ----------
        # prop drains: h0 -> ACT for all t; h1 -> DVE for t<6, ACT for t=6
        pouts = {}
        for t in range(NUM_ENT):
            ps_, pg_ = PROP_SG[t]
            pout = poutp.tile([H, B], BF16, tag="pout", name="pout")
            pouts[t] = pout
            for h in range(2):
                slot = psl.tile([H, HB], F32, tag="slot", name="pslot")
                for c in range(2):
                    nc.tensor.matmul(
                        slot[:, 512 * c:512 * c + 512],
                        wqimg[32 * ps_:32 * ps_ + 5, :],
                        prop6[32 * ps_:32 * ps_ + 5, pg_,
                              HB * h + 512 * c:HB * h + 512 * c + 512],
                        start=True, stop=True, tile_position=(32 * ps_, 0))
                if h == 1 and t < 6:
                    nc.vector.tensor_single_scalar(
                        pout[:, HB * h:HB * h + HB], slot[:], 0.0, op=ALU.max)
                else:
                    nc.scalar.activation(pout[:, HB * h:HB * h + HB],
                                         slot[:], AF.Relu)
            nc.sync.dma_start(out_d[0, t, :, :], pout[:])

            if t == 1:
                # squares + R matmuls + sqrt fit between prop units so the
                # PE stream stays dense while cmpb is fresh
                sq = stat.tile([42, B], BF16)
                nc.vector.tensor_mul(sq[0:42, :], cmpb[0:42, :], cmpb[0:42, :])
            if t == 2:
                distb = stat.tile([21, B], BF16)
                for h in range(2):
                    dslot = psl.tile([21, HB], F32, tag="slot", name="dslot")
                    for c in range(2):
                        nc.tensor.matmul(
                            dslot[:, 512 * c:512 * c + 512],
                            rm[0:42, :],
                            sq[0:42, HB * h + 512 * c:HB * h + 512 * c + 512],
                            start=True, stop=True, tile_position=(0, 0))
                    nc.scalar.activation(distb[:, HB * h:HB * h + HB],
                                         dslot[:], AF.Sqrt)
            if t == 3:
                # rhs6 staging: diffs from cmpb, dist rows, ones rows.
                # spread issue across sync/gpsimd/tensor queues.
                rhs6 = stat.tile([H, 6, B], BF16)
                qs = [nc.sync, nc.gpsimd, nc.gpsimd]
                qi = 0
                for s in range(4):
                    k0, np_ = STRIP_START[s], STRIP_NP[s]
                    for c in range(DIM_ENT):
                        qs[qi % 3].dma_start(
                            rhs6[32 * s + c:32 * s + c + 1, 0:np_, :],
                            cmpb[21 * c + k0:21 * c + k0 + np_, :])
                        qi += 1
                    qs[qi % 3].dma_start(
                        rhs6[32 * s + 4:32 * s + 5, 0:np_, :],
                        distb[k0:k0 + np_, :])
                    qi += 1
                    qs[qi % 3].dma_start(
                        rhs6[32 * s + 5:32 * s + 6, 0:6, :], onesb[0:6, :])
                    qi += 1

        # ---------- pair waves ----------
        def pair_mm_chunk(t, j, h, c, slot):
            a, b_ = (t, j) if t < j else (j, t)
            s, g = PAIR_SG[PAIR_IDX[(a, b_)]]
            img = wpimg if t < j else wmimg
            nc.tensor.matmul(
                slot[:, 512 * c:512 * c + 512],
                img[32 * s:32 * s + 6, :],
                rhs6[32 * s:32 * s + 6, g,
                     HB * h + 512 * c:HB * h + 512 * c + 512],
                start=True, stop=True, tile_position=(32 * s, 0))

        # global stream: per target [p0 A, p1 D, p2 A, p3 D, p4 A, p5 D];
        # chain in1 = r(p0); combine c_h = r(p0)+r(p2) per half... combine
        # uses r1=r(p2), r2=r(p4); in1 = r(p0).
        # stream items: (t, j, role, widx) — role A=act-relu, D=dve-stt.
        stream = []
        tstate = {}
        for t in range(NUM_ENT):
            order = _ordered_pairs(t)
            tstate[t] = {
                "order": order,
                "acc": accp.tile([H, B], BF16, tag="acc", name="acc"),
                "r": {}, "cb": None, "ndve": 0, "nact": 0,
            }
            for w in range(6):
                stream.append((t, order[w], "A" if w % 2 == 0 else "D", w))

        def drain_unit(t, j, role, w, h, slot):
            st = tstate[t]
            if role == "A":
                idx = w // 2            # 0,1,2 for w=0,2,4
                if h == 0:
                    st["r"][idx] = rp.tile([H, B], BF16, tag="r", name="r")
                r = st["r"][idx]
                nc.scalar.activation(r[:, HB * h:HB * h + HB], slot[:],
                                     AF.Relu)
            else:
                acc = st["acc"]
                in1 = (st["r"][0][:, HB * h:HB * h + HB] if w == 1
                       else acc[:, HB * h:HB * h + HB])
                nc.vector.scalar_tensor_tensor(
                    acc[:, HB * h:HB * h + HB], slot[:], 0.0, in1,
                    op0=ALU.max, op1=ALU.add)

        def post_unit(t, w, h):
            """combines/folds once the needed inputs exist."""
            st = tstate[t]
            if w == 4 and role_of(w) == "A":
                # r(p4) half h ready -> combine c_h = r1_h + r2_h
                if st["cb"] is None:
                    st["cb"] = cbp.tile([H, B], BF16, tag="c1", name="c1")
                cb = st["cb"]
                nc.gpsimd.tensor_add(cb[:, HB * h:HB * h + HB],
                                     st["r"][1][:, HB * h:HB * h + HB],
                                     st["r"][2][:, HB * h:HB * h + HB])
            if w == 5:
                # chain half h finished -> fold + (after both halves) DMA
                acc, cb = st["acc"], st["cb"]
                eng = nc.vector if t == 6 else nc.gpsimd
                if t == 6:
                    nc.vector.tensor_add(acc[:, HB * h:HB * h + HB],
                                         acc[:, HB * h:HB * h + HB],
                                         cb[:, HB * h:HB * h + HB])
                else:
                    nc.gpsimd.tensor_add(acc[:, HB * h:HB * h + HB],
                                         acc[:, HB * h:HB * h + HB],
                                         cb[:, HB * h:HB * h + HB])
                if h == 1:
                    nc.sync.dma_start(out_d[1, t, :, :], acc[:])

        def role_of(w):
            return "A" if w % 2 == 0 else "D"

        # r(p4)'s relu must precede the combine; note w=4 is role A.
        # Emit in 4-item waves, h0 then h1, MMs chunk-interleaved.
        for w0 in range(0, len(stream), 4):
            wave = stream[w0:w0 + 4]
            for h in range(2):
                slots = []
                for (t, j, role, w) in wave:
                    slots.append(psl.tile([H, HB], F32, tag="slot",
                                          name="wslot"))
                for c in range(2):
                    for (item, slot) in zip(wave, slots):
                        t, j, role, w = item
                        pair_mm_chunk(t, j, h, c, slot)
                for (item, slot) in zip(wave, slots):
                    t, j, role, w = item
                    drain_unit(t, j, role, w, h, slot)
                for (item, slot) in zip(wave, slots):
                    t, j, role, w = item
                    post_unit(t, w, h)

    nc.compile()
    return nc


_NC_CACHE = None


def _get_nc():
    global _NC_CACHE
    if _NC_CACHE is None:
        _NC_CACHE = build()
    return _NC_CACHE


def run(ctx, w_prop, b_prop, w_rel, b_rel, trace=False):
    bf = ml_dtypes.bfloat16
    ctx = np.asarray(ctx, dtype=np.float32)
    nc = _get_nc()
    shared = build_constants(np.asarray(w_prop, np.float32),
                             np.asarray(b_prop, np.float32),
                             np.asarray(w_rel, np.float32),
                             np.asarray(b_rel, np.float32))
    in_maps = []
    for c in range(N_CORES):
        m = dict(shared)
        m["ctxb"] = np.ascontiguousarray(ctx[:, c * B:(c + 1) * B]).astype(bf)
        in_maps.append(m)
    res = run_bass_kernel_spmd(nc, in_maps, core_ids=list(range(N_CORES)),
                               trace=trace)
    shards = [np.asarray(res.results[c]["out"]).astype(np.float32)
              for c in range(N_CORES)]
    full = np.concatenate(shards, axis=3)                     # [2,7,128,16384]
    out = np.transpose(full, (3, 1, 0, 2)).reshape(B_TOTAL, NUM_ENT, 2 * H)
    return np.ascontiguousarray(out), res


def kernel(ctx, w_prop, b_prop, w_rel, b_rel):
    return run(ctx, w_prop, b_prop, w_rel, b_rel)[0]
